# revision 1
# baseline (speedup 1.0000x reference)
"""Trainium2 Bass kernel for a 2-layer cross-attention dense transformer.

Sharding: data-parallel over batch — B=8 batch elements, one per NeuronCore.
Each core runs the full 2-layer transformer on its batch element; weights are
replicated. No collectives.

Per-core layout choices:
  - Activations are kept FEATURE-major in SBUF: x_fm[p, blk, l] = x[blk*128+p, l].
    All matmuls contract over features, so no transposes are needed anywhere.
  - Attention scores are computed transposed (scores_T[k_pos, q]); softmax
    denominators are obtained by appending a ones-column to the V tile in the
    attn@V matmul (M=65 per head), then dividing at PSUM eviction.
  - No max-subtraction in softmax (scores are O(5); exp is safe in fp32).
  - Matmuls run in float32r (single-pass fp32; ~TF32 input rounding). PSUM fp32.
  - LayerNorm partition sums via ones-vector matmuls, accumulated to full-L
    row tiles; rsqrt = exp(-0.5*ln(v+eps)) ONCE per layer-norm site so the
    ACT table only reloads a few times per layer.
"""

import numpy as np

# ---------------------------------------------------------------- constants
B, D, L0 = 8, 512, 1024
L = L0 + 1            # 1025 tokens (cls + 1024)
H, DH = 8, 64
DFF = 2048
NLAYER = 2
EPS = 1e-6
SCALE = 1.0 / (DH ** 0.5)

P = 128
DB = D // P           # 4 feature blocks
FB = DFF // P         # 16 dff blocks
LB = 9                # l-tiles over padded length (8 full + 2 rows)

N_CORES = 8


def _chunks(total, width):
    out, o = [], 0
    while o < total:
        w = min(width, total - o)
        out.append((o, w))
        o += w
    return out


LP = 1026                  # padded length: 3 even chunks of 342, no edges
QC = [(0, 342), (342, 342), (684, 342)]

# params tile slot indices (free-dim column j of the [128, NPARAM] tile)
BQ, BK, BV, BO, B2S = 0, 4, 8, 12, 16
LN1G, LN1B, LN2G, LN2B = 20, 24, 28, 32
B1S = 36
NPARAM = 52

_CACHE = {}


# ---------------------------------------------------------------- bass build
def _build_nc():
    import concourse.bass as bass
    import concourse.bacc as bacc
    import concourse.tile as tile
    from concourse import mybir
    from concourse.masks import make_identity

    f32 = mybir.dt.float32
    f32r = mybir.dt.float32r
    AO = mybir.AluOpType
    AF = mybir.ActivationFunctionType

    nc = bacc.Bacc("TRN2", target_bir_lowering=False, debug=False)

    # ---- DRAM I/O (per core) ----
    e1 = nc.dram_tensor("e1", [D, L0], f32, kind="ExternalInput")
    e2 = nc.dram_tensor("e2", [D, L0], f32, kind="ExternalInput")
    cls_t = nc.dram_tensor("cls", [D, 1], f32, kind="ExternalInput")
    wqT = nc.dram_tensor("wqT", [NLAYER, D, D], f32, kind="ExternalInput")
    wkT = nc.dram_tensor("wkT", [NLAYER, D, D], f32, kind="ExternalInput")
    wvT = nc.dram_tensor("wvT", [NLAYER, D, D], f32, kind="ExternalInput")
    woTh = nc.dram_tensor("woTh", [NLAYER, DH, H, D], f32, kind="ExternalInput")
    w1T = nc.dram_tensor("w1T", [NLAYER, D, DFF], f32, kind="ExternalInput")
    w2T = nc.dram_tensor("w2T", [NLAYER, DFF, D], f32, kind="ExternalInput")
    params_d = nc.dram_tensor("params", [NLAYER, P, NPARAM], f32,
                              kind="ExternalInput")
    bvrow_d = nc.dram_tensor("bvrow", [NLAYER, D], f32, kind="ExternalInput")
    out_d = nc.dram_tensor("out", [L, D], f32, kind="ExternalOutput")

    def r(ap):
        return ap.bitcast(f32r)

    def mm(out, lhsT, rhs, start, stop, n):
        # fp32r matmuls fail ISA checks for 1-wide moving operands; fall back
        # to plain fp32 there (edge chunks only).
        if n == 1:
            nc.tensor.matmul(out, lhsT.bitcast(f32), rhs.bitcast(f32),
                             start=start, stop=stop)
        else:
            nc.tensor.matmul(out, lhsT, rhs, start=start, stop=stop)

    with tile.TileContext(nc) as tc:
        with tc.tile_pool(name="persist", bufs=1) as pp, \
             tc.tile_pool(name="xpool", bufs=1) as xp, \
             tc.tile_pool(name="parms", bufs=2) as prm_pool:

            ones_f32 = pp.tile([P, P], f32)
            nc.vector.memset(ones_f32[:], 1.0)
            ones_col = pp.tile([P, 1], f32r)
            nc.vector.tensor_copy(ones_col[:], ones_f32[:, 0:1])
            ones_row = pp.tile([1, P], f32r)
            nc.vector.tensor_copy(ones_row[:], ones_f32[0:1, :])
            ident = pp.tile([P, P], f32)
            make_identity(nc, ident[:])
            eps_row = pp.tile([1, 1], f32)
            nc.vector.memset(eps_row[:], EPS)

            x = xp.tile([P, DB, LP], f32r, tag="x")
            x2 = xp.tile([P, DB, LP], f32r, tag="x2")
            xmid = xp.tile([P, DB, LP], f32r, tag="xmid")

            for m in range(DB):
                nc.sync.dma_start(x[:, m, 1:L], r(e1[m * P:(m + 1) * P, :]))
                nc.sync.dma_start(x2[:, m, 1:L], r(e2[m * P:(m + 1) * P, :]))
                nc.sync.dma_start(x[:, m, 0:1], r(cls_t[m * P:(m + 1) * P, :]))
                nc.sync.dma_start(x2[:, m, 0:1], r(cls_t[m * P:(m + 1) * P, :]))
            # zero the single pad token column (keeps everything finite)
            nc.vector.tensor_scalar_mul(x[:, :, L], ones_f32[:, 0:DB], 0.0)
            nc.vector.tensor_scalar_mul(x2[:, :, L], ones_f32[:, 0:DB], 0.0)

            def layernorm(psp, pstag, rows, sqpool, sqtag, src, prms,
                          GSLOT, BSLOT, out_ap):
                """LN over features of src [P, DB, L] -> out_ap (may alias).

                Row stats accumulate across all q-chunks first, the rsqrt
                runs once per site, then the affine applies per chunk.
                """
                m_full = rows.tile([1, LP], f32r, tag="lnm")
                s_full = rows.tile([1, LP], f32, tag="lns")
                for (qo, qw) in QC:
                    sq = sqpool.tile([P, DB, 512], f32r, tag=sqtag)
                    nc.vector.tensor_mul(sq[:, :, :qw], src[:, :, qo:qo + qw],
                                         src[:, :, qo:qo + qw])
                    mp = psp.tile([P, 512], f32, tag=pstag)
                    for kt in range(DB):
                        mm(mp[0:1, :qw], ones_col[:, 0:1],
                           src[:, kt, qo:qo + qw], kt == 0, kt == DB - 1, qw)
                    sp = psp.tile([P, 512], f32, tag=pstag)
                    for kt in range(DB):
                        mm(sp[0:1, :qw], ones_col[:, 0:1], sq[:, kt, :qw],
                           kt == 0, kt == DB - 1, qw)
                    nc.vector.tensor_scalar_mul(m_full[:, qo:qo + qw],
                                                mp[0:1, :qw], 1.0 / D)
                    nc.vector.tensor_scalar_mul(s_full[:, qo:qo + qw],
                                                sp[0:1, :qw], 1.0 / D)
                # var = E[x^2] - mean^2 ; rstd = exp(-0.5*ln(var+eps))
                msq = rows.tile([1, LP], f32, tag="lnt")
                nc.vector.tensor_mul(msq[:, :], m_full[:, :].bitcast(f32),
                                     m_full[:, :].bitcast(f32))
                nc.vector.tensor_sub(s_full[:, :], s_full[:, :], msq[:, :])
                nc.scalar.activation(s_full[:, :], s_full[:, :], AF.Ln,
                                     bias=eps_row[:])
                nc.vector.tensor_scalar_mul(s_full[:, :], s_full[:, :], -0.5)
                r_full = rows.tile([1, LP], f32r, tag="lnr")
                nc.scalar.activation(r_full[:, :], s_full[:, :], AF.Exp)
                for (qo, qw) in QC:
                    mb = psp.tile([P, 512], f32, tag=pstag)
                    mm(mb[:, :qw], ones_row[0:1, :], m_full[0:1, qo:qo + qw],
                       True, True, qw)
                    rb = psp.tile([P, 512], f32, tag=pstag)
                    mm(rb[:, :qw], ones_row[0:1, :], r_full[0:1, qo:qo + qw],
                       True, True, qw)
                    sq = sqpool.tile([P, DB, 512], f32r, tag=sqtag)
                    for m in range(DB):
                        nc.vector.tensor_sub(sq[:, m, :qw],
                                             src[:, m, qo:qo + qw], mb[:, :qw])
                        nc.vector.tensor_mul(sq[:, m, :qw], sq[:, m, :qw],
                                             rb[:, :qw])
                        nc.scalar.activation(
                            out_ap[:, m, qo:qo + qw], sq[:, m, :qw],
                            AF.Identity,
                            bias=prms[:, BSLOT + m:BSLOT + m + 1],
                            scale=prms[:, GSLOT + m:GSLOT + m + 1])

            for l in range(NLAYER):
                prms = prm_pool.tile([P, NPARAM], f32, tag="prms")
                nc.sync.dma_start(prms[:], params_d[l, :, :])

                # =================== PHASE A: attention ===================
                with tc.tile_pool(name=f"wA{l}", bufs=1) as wp, \
                     tc.tile_pool(name=f"woA{l}", bufs=1) as wop, \
                     tc.tile_pool(name=f"kvA{l}", bufs=1) as ap1, \
                     tc.tile_pool(name=f"exA{l}", bufs=9) as exl, \
                     tc.tile_pool(name=f"sbA{l}", bufs=1) as ap3, \
                     tc.tile_pool(name=f"rwA{l}", bufs=1) as rows, \
                     tc.tile_pool(name=f"raA{l}", bufs=4) as rrows, \
                     tc.tile_pool(name=f"psA{l}", bufs=4, space="PSUM") as psA, \
                     tc.tile_pool(name=f"psC{l}", bufs=2, space="PSUM") as psC, \
                     tc.tile_pool(name=f"psW{l}", bufs=2, space="PSUM") as psW:

                    # ---- K projection (full L); softmax scale folded in ----
                    wk_sb = wp.tile([P, DB, D], f32r, tag="w")
                    nc.sync.dma_start(
                        wk_sb[:],
                        r(wkT[l, :, :].rearrange("(b p) n -> p b n", p=P)))
                    K_fm = ap1.tile([P, DB, LP], f32r, tag="K")
                    for m in range(DB):
                        for (o, w) in QC:
                            kp = psA.tile([P, 512], f32, tag="psA")
                            for kt in range(DB):
                                mm(kp[:, :w],
                                   wk_sb[:, kt, m * P:(m + 1) * P],
                                   x2[:, kt, o:o + w],
                                   kt == 0, kt == DB - 1, w)
                            nc.vector.tensor_scalar(
                                out=K_fm[:, m, o:o + w], in0=kp[:, :w],
                                scalar1=prms[:, BK + m:BK + m + 1],
                                scalar2=SCALE, op0=AO.add, op1=AO.mult)

                    # ---- V projection (token-major, ones column at DH) ----
                    wv_sb = wp.tile([P, DB, D], f32r, tag="w")
                    nc.sync.dma_start(
                        wv_sb[:],
                        r(wvT[l, :, :].rearrange("(b p) n -> p b n", p=P)))
                    bvb = ap1.tile([P, D], f32, tag="bvb")
                    nc.sync.dma_start(
                        bvb[:],
                        bass.AP(tensor=bvrow_d, offset=l * D,
                                ap=[[0, P], [1, D]]))
                    V_tm = ap1.tile([P, LB, H, DH + 1], f32r, tag="V")
                    nc.vector.tensor_copy(
                        V_tm[:, :, :, DH],
                        ones_f32[:, 0:LB * H].rearrange("p (a b) -> p a b",
                                                        a=LB))
                    for mt in range(LB):
                        nrow = P if mt < LB - 1 else L - (LB - 1) * P
                        vp = psA.tile([P, 512], f32, tag="psA")
                        for kt in range(DB):
                            nc.tensor.matmul(
                                vp[:nrow, :D],
                                x2[:, kt, mt * P:mt * P + nrow],
                                wv_sb[:, kt, :],
                                start=(kt == 0), stop=(kt == DB - 1))
                        nc.vector.tensor_tensor(
                            out=V_tm[:nrow, mt, :, 0:DH],
                            in0=vp[:nrow, :D].rearrange("p (h c) -> p h c", h=H),
                            in1=bvb[:nrow, :].rearrange("p (h c) -> p h c", h=H),
                            op=AO.add)

                    # ---- wq & wo loads ----
                    wq_sb = wp.tile([P, DB, D], f32r, tag="w")
                    nc.sync.dma_start(
                        wq_sb[:],
                        r(wqT[l, :, :].rearrange("(b p) n -> p b n", p=P)))
                    wo_sb = wop.tile([DH, H, D], f32r, tag="wo")
                    nc.sync.dma_start(wo_sb[:], r(woTh[l, :, :, :]))

                    # ---- per q-chunk attention ----
                    for (qo, qw) in QC:
                        Q_fm = ap3.tile([P, DB, 512], f32r, tag="Q")
                        for m in range(DB):
                            qp = psA.tile([P, 512], f32, tag="psA")
                            for kt in range(DB):
                                mm(qp[:, :qw],
                                   wq_sb[:, kt, m * P:(m + 1) * P],
                                   x[:, kt, qo:qo + qw],
                                   kt == 0, kt == DB - 1, qw)
                            nc.vector.tensor_scalar_add(
                                Q_fm[:, m, :qw], qp[:, :qw],
                                prms[:, BQ + m:BQ + m + 1])

                        ctx_sb = ap3.tile([DH, H, 512], f32r, tag="ctx")

                        for hp in range(H // 2):
                            h0, h1 = 2 * hp, 2 * hp + 1
                            ets = {h0: [], h1: []}
                            for kt in range(LB):
                                nrow = P if kt < LB - 1 else L - (LB - 1) * P
                                for h in (h0, h1):
                                    base, blk = (h % 2) * DH, h // 2
                                    sp = psA.tile([P, 512], f32, tag="psA")
                                    mm(sp[:nrow, :qw],
                                       K_fm[base:base + DH, blk,
                                            kt * P:kt * P + nrow],
                                       Q_fm[base:base + DH, blk, :qw],
                                       True, True, qw)
                                    et = exl.tile([P, 512], f32r, tag="exp")
                                    nc.scalar.activation(et[:nrow, :qw],
                                                         sp[:nrow, :qw],
                                                         AF.Exp)
                                    ets[h].append(et)
                            for h in (h0, h1):
                                cp = psC.tile([DH + 1, 512], f32, tag="psC")
                                for kt in range(LB):
                                    nrow = (P if kt < LB - 1
                                            else L - (LB - 1) * P)
                                    mm(cp[:, :qw],
                                       V_tm[:nrow, kt, h, :],
                                       ets[h][kt][:nrow, :qw],
                                       kt == 0, kt == LB - 1, qw)
                                # normalize by the ones-column denominators
                                drow = rrows.tile([1, 512], f32, tag="row")
                                nc.vector.tensor_copy(drow[:, :qw],
                                                      cp[DH:DH + 1, :qw])
                                rrow = rrows.tile([1, 512], f32, tag="row")
                                nc.vector.reciprocal_approx_fast(
                                    rrow[:, :qw], drow[:, :qw])
                                rrowr = rrows.tile([1, 512], f32r, tag="row")
                                nc.vector.tensor_copy(rrowr[:, :qw],
                                                      rrow[:, :qw])
                                rb = psA.tile([P, 512], f32, tag="psA")
                                mm(rb[:DH, :qw], ones_row[0:1, 0:DH],
                                   rrowr[0:1, :qw], True, True, qw)
                                nc.vector.tensor_copy(ctx_sb[:, h, :qw],
                                                      cp[:DH, :qw])
                                nc.vector.tensor_tensor(
                                    out=ctx_sb[:, h, :qw],
                                    in0=ctx_sb[:, h, :qw],
                                    in1=rb[:DH, :qw], op=AO.mult)

                        # ---- output projection + bo + residual -> xmid ----
                        for m in range(DB):
                            op_ = psW.tile([P, 512], f32, tag="psW")
                            for h in range(H):
                                mm(op_[:, :qw],
                                   wo_sb[:, h, m * P:(m + 1) * P],
                                   ctx_sb[:, h, :qw],
                                   h == 0, h == H - 1, qw)
                            nc.vector.scalar_tensor_tensor(
                                out=xmid[:, m, qo:qo + qw], in0=op_[:, :qw],
                                scalar=prms[:, BO + m:BO + m + 1],
                                in1=x[:, m, qo:qo + qw],
                                op0=AO.add, op1=AO.add)

                    # ---- LN1 (in place on xmid) ----
                    layernorm(psA, "psA", rows, ap3, "sq", xmid, prms,
                              LN1G, LN1B, xmid)

                # =================== PHASE B: FFN ===================
                with tc.tile_pool(name=f"wB{l}", bufs=1) as fwp, \
                     tc.tile_pool(name=f"hB{l}", bufs=1) as fhp, \
                     tc.tile_pool(name=f"sqB{l}", bufs=1) as fsq, \
                     tc.tile_pool(name=f"rwB{l}", bufs=1) as rowsB, \
                     tc.tile_pool(name=f"psH{l}", bufs=4, space="PSUM") as psH, \
                     tc.tile_pool(name=f"psF{l}", bufs=2, space="PSUM") as psF:

                    w1_sb = fwp.tile([P, DB, DFF], f32r, tag="w1")
                    nc.sync.dma_start(
                        w1_sb[:],
                        r(w1T[l, :, :].rearrange("(b p) n -> p b n", p=P)))
                    w2_sb = fwp.tile([P, FB, D], f32r, tag="w2")
                    nc.sync.dma_start(
                        w2_sb[:],
                        r(w2T[l, :, :].rearrange("(b p) n -> p b n", p=P)))

                    for (qo, qw) in QC:
                        h_sb = fhp.tile([P, FB, 512], f32r, tag="h")
                        for mf in range(FB):
                            hp = psH.tile([P, 512], f32, tag="psH")
                            for kt in range(DB):
                                mm(hp[:, :qw],
                                   w1_sb[:, kt, mf * P:(mf + 1) * P],
                                   xmid[:, kt, qo:qo + qw],
                                   kt == 0, kt == DB - 1, qw)
                            nc.scalar.activation(
                                h_sb[:, mf, :qw], hp[:, :qw], AF.Gelu,
                                bias=prms[:, B1S + mf:B1S + mf + 1])
                        for m in range(DB):
                            fp = psF.tile([P, 512], f32, tag="psF")
                            for kt in range(FB):
                                mm(fp[:, :qw],
                                   w2_sb[:, kt, m * P:(m + 1) * P],
                                   h_sb[:, kt, :qw],
                                   kt == 0, kt == FB - 1, qw)
                            nc.vector.scalar_tensor_tensor(
                                out=x[:, m, qo:qo + qw], in0=fp[:, :qw],
                                scalar=prms[:, B2S + m:B2S + m + 1],
                                in1=xmid[:, m, qo:qo + qw],
                                op0=AO.add, op1=AO.add)

                    # ---- LN2 (in place on x) ----
                    layernorm(psH, "psH", rowsB, fsq, "sq", x, prms,
                              LN2G, LN2B, x)

            # =================== transpose x -> out ===================
            with tc.tile_pool(name="psT", bufs=4, space="PSUM") as psT, \
                 tc.tile_pool(name="sbT", bufs=4) as sbT:
                for mt in range(LB):
                    nrow = P if mt < LB - 1 else L - (LB - 1) * P
                    for m in range(DB):
                        tp = psT.tile([P, P], f32, tag="psT")
                        nc.tensor.transpose(
                            tp[:nrow, :],
                            x[:, m, mt * P:mt * P + nrow].bitcast(f32),
                            ident[:])
                        ts = sbT.tile([P, P], f32, tag="sbT")
                        nc.vector.tensor_copy(ts[:nrow, :], tp[:nrow, :])
                        nc.sync.dma_start(
                            out_d[mt * P:mt * P + nrow, m * P:(m + 1) * P],
                            ts[:nrow, :])

    nc.compile()
    return nc


# ---------------------------------------------------------------- host side
def _prep_inputs(inputs):
    f = np.float32
    Wq, Wk, Wv, Wo = inputs["Wq"], inputs["Wk"], inputs["Wv"], inputs["Wo"]
    W1, W2 = inputs["W1"], inputs["W2"]

    wqT = np.ascontiguousarray(np.transpose(np.asarray(Wq, f), (0, 2, 1)))
    wkT = np.ascontiguousarray(np.transpose(np.asarray(Wk, f), (0, 2, 1)))
    wvT = np.ascontiguousarray(np.transpose(np.asarray(Wv, f), (0, 2, 1)))
    w1T = np.ascontiguousarray(np.transpose(np.asarray(W1, f), (0, 2, 1)))
    w2T = np.ascontiguousarray(np.transpose(np.asarray(W2, f), (0, 2, 1)))
    woTh = np.ascontiguousarray(
        np.transpose(np.asarray(Wo, f).reshape(NLAYER, D, H, DH), (0, 3, 2, 1)))

    def col(v):  # [NLAYER, D] -> [NLAYER, P, DB]
        return np.transpose(np.asarray(v, f).reshape(NLAYER, DB, P), (0, 2, 1))

    params = np.zeros((NLAYER, P, NPARAM), f)
    params[:, :, BQ:BQ + DB] = col(inputs["bq"])
    params[:, :, BK:BK + DB] = col(inputs["bk"])
    params[:, :, BV:BV + DB] = col(inputs["bv"])
    params[:, :, BO:BO + DB] = col(inputs["bo"])
    params[:, :, B2S:B2S + DB] = col(inputs["b2"])
    params[:, :, LN1G:LN1G + DB] = col(inputs["ln1_g"])
    params[:, :, LN1B:LN1B + DB] = col(inputs["ln1_b"])
    params[:, :, LN2G:LN2G + DB] = col(inputs["ln2_g"])
    params[:, :, LN2B:LN2B + DB] = col(inputs["ln2_b"])
    params[:, :, B1S:B1S + FB] = np.transpose(
        np.asarray(inputs["b1"], f).reshape(NLAYER, FB, P), (0, 2, 1))

    shared = {
        "cls": np.ascontiguousarray(
            np.asarray(inputs["cls_token"], f).reshape(D, 1)),
        "wqT": wqT, "wkT": wkT, "wvT": wvT, "woTh": woTh,
        "w1T": w1T, "w2T": w2T, "params": params,
        "bvrow": np.ascontiguousarray(np.asarray(inputs["bv"], f)),
    }
    e1 = np.asarray(inputs["embed1"], f)
    e2 = np.asarray(inputs["embed2"], f)
    in_maps = []
    for b in range(N_CORES):
        m = dict(shared)
        m["e1"] = np.ascontiguousarray(e1[b])
        m["e2"] = np.ascontiguousarray(e2[b])
        in_maps.append(m)
    return in_maps


def _run(inputs, trace=False, **kw):
    from concourse.bass_utils import run_bass_kernel_spmd

    if "nc" not in _CACHE:
        _CACHE["nc"] = _build_nc()
    nc = _CACHE["nc"]
    in_maps = _prep_inputs(inputs)
    res = run_bass_kernel_spmd(nc, in_maps, list(range(N_CORES)), trace=trace,
                               **kw)
    out = np.stack([res.results[b]["out"] for b in range(N_CORES)], axis=0)
    return out.astype(np.float32), res


def kernel(**inputs):
    out, _ = _run(inputs, trace=False)
    return out



# revision 5
# speedup vs baseline: 1.4844x; 1.4844x over previous
"""Trainium2 Bass kernel for a 2-layer cross-attention dense transformer.

Sharding: data-parallel over batch — B=8 batch elements, one per NeuronCore.
Each core runs the full 2-layer transformer on its batch element; weights are
replicated. No collectives.

v2 layout/schedule choices (vs v1 baseline at 931us):
  - All matmul operands in bf16 (fp32 PSUM accumulation). Halves weight DMA,
    enables 2x/4x DVE modes, same 1 cycle/row PE rate as f32r.
  - Scores for a head PAIR go into one 2-bank PSUM tile; ONE exp ACTIVATE
    covers both heads (684 cols), halving ScalarE instruction overheads.
  - exp pool holds 18 pair-tiles so attnV(pair p) never starves exp(p+1);
    scores for pair p+1 are emitted BEFORE attnV(p) to keep PE busy while
    ScalarE catches up.
  - LN affine runs on DVE (tensor ops + two-scalar tensor_scalar), keeping
    ScalarE for exp/gelu only.
  - All weight DMAs for a layer are issued at layer start so W1/W2 land
    during attention.
"""

import numpy as np

# ---------------------------------------------------------------- constants
B, D, L0 = 8, 512, 1024
L = L0 + 1            # 1025 tokens (cls + 1024)
H, DH = 8, 64
DFF = 2048
NLAYER = 2
EPS = 1e-6
SCALE = 1.0 / (DH ** 0.5)

P = 128
DB = D // P           # 4 feature blocks
FB = DFF // P         # 16 dff blocks
LB = 9                # l-tiles over length (8 full + 1 row)

N_CORES = 8

LP = 1026                  # padded length: 3 even chunks of 342, no edges
QC = [(0, 342), (342, 342), (684, 342)]

# params tile slot indices (free-dim column j of the [128, NPARAM] tile)
BQ, BK, BV, BO, B2S = 0, 4, 8, 12, 16
LN1G, LN1B, LN2G, LN2B = 20, 24, 28, 32
B1S = 36
NPARAM = 52

_CACHE = {}


# ---------------------------------------------------------------- bass build
def _build_nc():
    import concourse.bass as bass
    import concourse.bacc as bacc
    import concourse.tile as tile
    from concourse import mybir
    from concourse.masks import make_identity

    f32 = mybir.dt.float32
    bf16 = mybir.dt.bfloat16
    AO = mybir.AluOpType
    AF = mybir.ActivationFunctionType

    nc = bacc.Bacc("TRN2", target_bir_lowering=False, debug=False)

    # ---- DRAM I/O (per core) ----
    e1 = nc.dram_tensor("e1", [D, L0], bf16, kind="ExternalInput")
    e2 = nc.dram_tensor("e2", [D, L0], bf16, kind="ExternalInput")
    cls_t = nc.dram_tensor("cls", [D, 1], bf16, kind="ExternalInput")
    wqT = nc.dram_tensor("wqT", [NLAYER, D, D], bf16, kind="ExternalInput")
    wkT = nc.dram_tensor("wkT", [NLAYER, D, D], bf16, kind="ExternalInput")
    wvT = nc.dram_tensor("wvT", [NLAYER, D, D], bf16, kind="ExternalInput")
    woTh = nc.dram_tensor("woTh", [NLAYER, DH, H, D], bf16,
                          kind="ExternalInput")
    w1T = nc.dram_tensor("w1T", [NLAYER, D, DFF], bf16, kind="ExternalInput")
    w2T = nc.dram_tensor("w2T", [NLAYER, DFF, D], bf16, kind="ExternalInput")
    params_d = nc.dram_tensor("params", [NLAYER, P, NPARAM], f32,
                              kind="ExternalInput")
    bvrow_d = nc.dram_tensor("bvrow", [NLAYER, D], f32, kind="ExternalInput")
    out_d = nc.dram_tensor("out", [L, D], f32, kind="ExternalOutput")

    with tile.TileContext(nc) as tc:
        with tc.tile_pool(name="persist", bufs=1) as pp, \
             tc.tile_pool(name="xpool", bufs=1) as xp, \
             tc.tile_pool(name="parms", bufs=2) as prm_pool:

            ones_bf = pp.tile([P, P], bf16)
            nc.vector.memset(ones_bf[:], 1.0)
            ones_col = pp.tile([P, 1], bf16)
            nc.vector.tensor_copy(ones_col[:], ones_bf[:, 0:1])
            ones_row = pp.tile([1, P], bf16)
            nc.vector.tensor_copy(ones_row[:], ones_bf[0:1, :])
            ident = pp.tile([P, P], bf16)
            make_identity(nc, ident[:])
            eps_row = pp.tile([1, 1], f32)
            nc.vector.memset(eps_row[:], EPS)

            x = xp.tile([P, DB, LP], bf16, tag="x")
            x2 = xp.tile([P, DB, LP], bf16, tag="x2")
            xmid = xp.tile([P, DB, LP], bf16, tag="xmid")

            for m in range(DB):
                nc.sync.dma_start(x2[:, m, 1:L], e2[m * P:(m + 1) * P, :])
                nc.sync.dma_start(x2[:, m, 0:1], cls_t[m * P:(m + 1) * P, :])
            for m in range(DB):
                nc.sync.dma_start(x[:, m, 1:L], e1[m * P:(m + 1) * P, :])
                nc.sync.dma_start(x[:, m, 0:1], cls_t[m * P:(m + 1) * P, :])
            # zero the single pad token column (keeps everything finite)
            nc.vector.tensor_scalar_mul(x[:, :, L], ones_bf[:, 0:DB], 0.0)
            nc.vector.tensor_scalar_mul(x2[:, :, L], ones_bf[:, 0:DB], 0.0)

            def layernorm(psp, pstag, rows, sqpool, sqtag, src, prms,
                          GSLOT, BSLOT, out_ap):
                """LN over features of src [P, DB, L] -> out_ap (may alias).

                Row stats accumulate across all q-chunks first, the rsqrt
                runs once per site, then the affine applies per chunk on DVE.
                """
                m_full = rows.tile([1, LP], f32, tag="lnm")
                s_full = rows.tile([1, LP], f32, tag="lns")
                for (qo, qw) in QC:
                    sq = sqpool.tile([P, DB, 512], bf16, tag=sqtag)
                    nc.vector.tensor_mul(sq[:, :, :qw], src[:, :, qo:qo + qw],
                                         src[:, :, qo:qo + qw])
                    mp = psp.tile([P, 512], f32, tag=pstag)
                    for kt in range(DB):
                        nc.tensor.matmul(mp[0:1, :qw], ones_col[:, 0:1],
                                         src[:, kt, qo:qo + qw],
                                         start=(kt == 0), stop=(kt == DB - 1))
                    sp = psp.tile([P, 512], f32, tag=pstag)
                    for kt in range(DB):
                        nc.tensor.matmul(sp[0:1, :qw], ones_col[:, 0:1],
                                         sq[:, kt, :qw],
                                         start=(kt == 0), stop=(kt == DB - 1))
                    nc.vector.tensor_scalar_mul(m_full[:, qo:qo + qw],
                                                mp[0:1, :qw], 1.0 / D)
                    nc.vector.tensor_scalar_mul(s_full[:, qo:qo + qw],
                                                sp[0:1, :qw], 1.0 / D)
                # var = E[x^2] - mean^2 ; rstd = exp(-0.5*ln(var+eps))
                msq = rows.tile([1, LP], f32, tag="lnt")
                nc.vector.tensor_mul(msq[:, :], m_full[:, :], m_full[:, :])
                nc.vector.tensor_sub(s_full[:, :], s_full[:, :], msq[:, :])
                nc.scalar.activation(s_full[:, :], s_full[:, :], AF.Ln,
                                     bias=eps_row[:])
                nc.vector.tensor_scalar_mul(s_full[:, :], s_full[:, :], -0.5)
                r_full = rows.tile([1, LP], f32, tag="lnr")
                nc.scalar.activation(r_full[:, :], s_full[:, :], AF.Exp)
                m_bf = rows.tile([1, LP], bf16, tag="lnmb")
                nc.vector.tensor_copy(m_bf[:, :], m_full[:, :])
                r_bf = rows.tile([1, LP], bf16, tag="lnrb")
                nc.vector.tensor_copy(r_bf[:, :], r_full[:, :])
                for (qo, qw) in QC:
                    mb = psp.tile([P, 512], f32, tag=pstag)
                    nc.tensor.matmul(mb[:, :qw], ones_row[0:1, :],
                                     m_bf[0:1, qo:qo + qw],
                                     start=True, stop=True)
                    rb = psp.tile([P, 512], f32, tag=pstag)
                    nc.tensor.matmul(rb[:, :qw], ones_row[0:1, :],
                                     r_bf[0:1, qo:qo + qw],
                                     start=True, stop=True)
                    sq = sqpool.tile([P, DB, 512], bf16, tag=sqtag)
                    for m in range(DB):
                        nc.vector.tensor_sub(sq[:, m, :qw],
                                             src[:, m, qo:qo + qw], mb[:, :qw])
                        nc.vector.tensor_mul(sq[:, m, :qw], sq[:, m, :qw],
                                             rb[:, :qw])
                        nc.vector.tensor_scalar(
                            out=out_ap[:, m, qo:qo + qw], in0=sq[:, m, :qw],
                            scalar1=prms[:, GSLOT + m:GSLOT + m + 1],
                            scalar2=prms[:, BSLOT + m:BSLOT + m + 1],
                            op0=AO.mult, op1=AO.add)

            for l in range(NLAYER):
                prms = prm_pool.tile([P, NPARAM], f32, tag="prms")
                nc.sync.dma_start(prms[:], params_d[l, :, :])

                with tc.tile_pool(name=f"wA{l}", bufs=1) as wp, \
                     tc.tile_pool(name=f"woA{l}", bufs=1) as wop, \
                     tc.tile_pool(name=f"kvA{l}", bufs=1) as ap1, \
                     tc.tile_pool(name=f"exA{l}", bufs=18) as exl, \
                     tc.tile_pool(name=f"sbA{l}", bufs=1) as ap3, \
                     tc.tile_pool(name=f"fwB{l}", bufs=1) as fwp, \
                     tc.tile_pool(name=f"rwA{l}", bufs=1) as rows, \
                     tc.tile_pool(name=f"raA{l}", bufs=4) as rrows, \
                     tc.tile_pool(name=f"psA{l}", bufs=2, space="PSUM") as psA, \
                     tc.tile_pool(name=f"psC{l}", bufs=2, space="PSUM") as psC, \
                     tc.tile_pool(name=f"psW{l}", bufs=2, space="PSUM") as psW:

                    # ---- issue ALL layer weight DMAs up front ----
                    wk_sb = wp.tile([P, DB, D], bf16, tag="wk")
                    nc.sync.dma_start(
                        wk_sb[:],
                        wkT[l, :, :].rearrange("(b p) n -> p b n", p=P))
                    wv_sb = wp.tile([P, DB, D], bf16, tag="wv")
                    nc.sync.dma_start(
                        wv_sb[:],
                        wvT[l, :, :].rearrange("(b p) n -> p b n", p=P))
                    wq_sb = wp.tile([P, DB, D], bf16, tag="wq")
                    nc.sync.dma_start(
                        wq_sb[:],
                        wqT[l, :, :].rearrange("(b p) n -> p b n", p=P))
                    wo_sb = wop.tile([DH, H, D], bf16, tag="wo")
                    nc.sync.dma_start(wo_sb[:], woTh[l, :, :, :])
                    w1_sb = fwp.tile([P, DB, DFF], bf16, tag="w1")
                    nc.sync.dma_start(
                        w1_sb[:],
                        w1T[l, :, :].rearrange("(b p) n -> p b n", p=P))
                    w2_sb = fwp.tile([P, FB, D], bf16, tag="w2")
                    nc.sync.dma_start(
                        w2_sb[:],
                        w2T[l, :, :].rearrange("(b p) n -> p b n", p=P))
                    bvb = ap1.tile([P, D], f32, tag="bvb")
                    nc.sync.dma_start(
                        bvb[:],
                        bass.AP(tensor=bvrow_d, offset=l * D,
                                ap=[[0, P], [1, D]]))

                    # ---- K projection (full L); softmax scale folded in ----
                    K_fm = ap1.tile([P, DB, LP], bf16, tag="K")
                    for m in range(DB):
                        for (o, w) in QC:
                            kp = psA.tile([P, 2, 512], f32, tag="psA")
                            for kt in range(DB):
                                nc.tensor.matmul(
                                    kp[:, 0, :w],
                                    wk_sb[:, kt, m * P:(m + 1) * P],
                                    x2[:, kt, o:o + w],
                                    start=(kt == 0), stop=(kt == DB - 1))
                            nc.vector.tensor_scalar(
                                out=K_fm[:, m, o:o + w], in0=kp[:, 0, :w],
                                scalar1=prms[:, BK + m:BK + m + 1],
                                scalar2=SCALE, op0=AO.add, op1=AO.mult)

                    # ---- V projection (token-major, ones column at DH) ----
                    V_tm = ap1.tile([P, LB, H, DH + 1], bf16, tag="V")
                    nc.vector.tensor_copy(
                        V_tm[:, :, :, DH],
                        ones_bf[:, 0:LB * H].rearrange("p (a b) -> p a b",
                                                       a=LB))
                    for mt in range(LB):
                        nrow = P if mt < LB - 1 else L - (LB - 1) * P
                        vp = psA.tile([P, 2, 512], f32, tag="psA")
                        for kt in range(DB):
                            nc.tensor.matmul(
                                vp[:nrow, 0, :D],
                                x2[:, kt, mt * P:mt * P + nrow],
                                wv_sb[:, kt, :],
                                start=(kt == 0), stop=(kt == DB - 1))
                        nc.vector.tensor_tensor(
                            out=V_tm[:nrow, mt, :, 0:DH],
                            in0=vp[:nrow, 0, :D].rearrange("p (h c) -> p h c",
                                                           h=H),
                            in1=bvb[:nrow, :].rearrange("p (h c) -> p h c",
                                                        h=H),
                            op=AO.add)

                    # ---- per q-chunk attention ----
                    for ci, (qo, qw) in enumerate(QC):
                        Q_fm = ap3.tile([P, DB, 512], bf16, tag="Q")
                        for m in range(DB):
                            qp = psW.tile([P, 512], f32, tag="psW")
                            for kt in range(DB):
                                nc.tensor.matmul(
                                    qp[:, :qw],
                                    wq_sb[:, kt, m * P:(m + 1) * P],
                                    x[:, kt, qo:qo + qw],
                                    start=(kt == 0), stop=(kt == DB - 1))
                            nc.vector.tensor_scalar_add(
                                Q_fm[:, m, :qw], qp[:, :qw],
                                prms[:, BQ + m:BQ + m + 1])

                        ctx_sb = ap3.tile([DH, H, 512], bf16, tag="ctx")

                        def scores_pair(hp):
                            """Scores + exp for head pair hp; returns ets."""
                            h0 = 2 * hp
                            ets = []
                            for kt in range(LB):
                                nrow = P if kt < LB - 1 else L - (LB - 1) * P
                                sp = psA.tile([P, 2, 512], f32, tag="psA")
                                for j in range(2):
                                    b = ((h0 + j) % 2) * DH
                                    bl = (h0 + j) // 2
                                    nc.tensor.matmul(
                                        sp[:nrow, j, :qw],
                                        K_fm[b:b + DH, bl,
                                             kt * P:kt * P + nrow],
                                        Q_fm[b:b + DH, bl, :qw],
                                        start=True, stop=True)
                                et = exl.tile([P, 2, 512], bf16, tag="exp")
                                nc.scalar.activation(et[:nrow, :, :qw],
                                                     sp[:nrow, :, :qw],
                                                     AF.Exp)
                                ets.append(et)
                            return ets

                        def attnv_pair(hp, ets):
                            h0 = 2 * hp
                            for j in range(2):
                                h = h0 + j
                                cp = psC.tile([DH + 1, 512], f32, tag="psC")
                                for kt in range(LB):
                                    nrow = (P if kt < LB - 1
                                            else L - (LB - 1) * P)
                                    nc.tensor.matmul(
                                        cp[:, :qw],
                                        V_tm[:nrow, kt, h, :],
                                        ets[kt][:nrow, j, :qw],
                                        start=(kt == 0), stop=(kt == LB - 1))
                                # normalize by the ones-column denominators
                                drow = rrows.tile([1, 512], f32, tag="row")
                                nc.vector.tensor_copy(drow[:, :qw],
                                                      cp[DH:DH + 1, :qw])
                                rrow = rrows.tile([1, 512], f32, tag="row")
                                nc.vector.reciprocal_approx_fast(
                                    rrow[:, :qw], drow[:, :qw])
                                rrowr = rrows.tile([1, 512], bf16, tag="rowb")
                                nc.vector.tensor_copy(rrowr[:, :qw],
                                                      rrow[:, :qw])
                                rb = psW.tile([P, 512], f32, tag="psW")
                                nc.tensor.matmul(rb[:DH, :qw],
                                                 ones_row[0:1, 0:DH],
                                                 rrowr[0:1, :qw],
                                                 start=True, stop=True)
                                nc.vector.tensor_copy(ctx_sb[:, h, :qw],
                                                      cp[:DH, :qw])
                                nc.vector.tensor_tensor(
                                    out=ctx_sb[:, h, :qw],
                                    in0=ctx_sb[:, h, :qw],
                                    in1=rb[:DH, :qw], op=AO.mult)

                        # software pipeline: scores(p+1) before attnV(p)
                        ets_cur = scores_pair(0)
                        for hp in range(H // 2):
                            ets_next = (scores_pair(hp + 1)
                                        if hp + 1 < H // 2 else None)
                            attnv_pair(hp, ets_cur)
                            ets_cur = ets_next

                        # ---- output projection + bo + residual -> xmid ----
                        for m in range(DB):
                            op_ = psW.tile([P, 512], f32, tag="psW")
                            for h in range(H):
                                nc.tensor.matmul(
                                    op_[:, :qw],
                                    wo_sb[:, h, m * P:(m + 1) * P],
                                    ctx_sb[:, h, :qw],
                                    start=(h == 0), stop=(h == H - 1))
                            nc.vector.scalar_tensor_tensor(
                                out=xmid[:, m, qo:qo + qw], in0=op_[:, :qw],
                                scalar=prms[:, BO + m:BO + m + 1],
                                in1=x[:, m, qo:qo + qw],
                                op0=AO.add, op1=AO.add)

                    # ---- LN1 (in place on xmid) ----
                    layernorm(psW, "psW", rows, ap3, "sq", xmid, prms,
                              LN1G, LN1B, xmid)

                    # =================== PHASE B: FFN ===================
                    with tc.tile_pool(name=f"hB{l}", bufs=1) as fhp:
                        for (qo, qw) in QC:
                            h_sb = fhp.tile([P, FB, 512], bf16, tag="h")
                            for mf in range(FB):
                                hp_ = psA.tile([P, 2, 512], f32, tag="psA")
                                for kt in range(DB):
                                    nc.tensor.matmul(
                                        hp_[:, 0, :qw],
                                        w1_sb[:, kt, mf * P:(mf + 1) * P],
                                        xmid[:, kt, qo:qo + qw],
                                        start=(kt == 0), stop=(kt == DB - 1))
                                nc.scalar.activation(
                                    h_sb[:, mf, :qw], hp_[:, 0, :qw], AF.Gelu,
                                    bias=prms[:, B1S + mf:B1S + mf + 1])
                            for m in range(DB):
                                fp = psW.tile([P, 512], f32, tag="psW")
                                for kt in range(FB):
                                    nc.tensor.matmul(
                                        fp[:, :qw],
                                        w2_sb[:, kt, m * P:(m + 1) * P],
                                        h_sb[:, kt, :qw],
                                        start=(kt == 0), stop=(kt == FB - 1))
                                nc.vector.scalar_tensor_tensor(
                                    out=x[:, m, qo:qo + qw], in0=fp[:, :qw],
                                    scalar=prms[:, B2S + m:B2S + m + 1],
                                    in1=xmid[:, m, qo:qo + qw],
                                    op0=AO.add, op1=AO.add)

                        # ---- LN2 (in place on x) ----
                        layernorm(psW, "psW", rows, ap3, "sq", x, prms,
                                  LN2G, LN2B, x)

            # =================== transpose x -> out ===================
            with tc.tile_pool(name="psT", bufs=4, space="PSUM") as psT, \
                 tc.tile_pool(name="sbT", bufs=4) as sbT:
                for mt in range(LB):
                    nrow = P if mt < LB - 1 else L - (LB - 1) * P
                    for m in range(DB):
                        tp = psT.tile([P, P], bf16, tag="psT")
                        nc.tensor.transpose(
                            tp[:nrow, :],
                            x[:, m, mt * P:mt * P + nrow],
                            ident[:])
                        ts = sbT.tile([P, P], f32, tag="sbT")
                        nc.vector.tensor_copy(ts[:nrow, :], tp[:nrow, :])
                        nc.sync.dma_start(
                            out_d[mt * P:mt * P + nrow, m * P:(m + 1) * P],
                            ts[:nrow, :])

    nc.compile()
    return nc


# ---------------------------------------------------------------- host side
def _prep_inputs(inputs):
    import ml_dtypes
    f = np.float32
    bf = ml_dtypes.bfloat16
    Wq, Wk, Wv, Wo = inputs["Wq"], inputs["Wk"], inputs["Wv"], inputs["Wo"]
    W1, W2 = inputs["W1"], inputs["W2"]

    def tb(a, perm):
        return np.ascontiguousarray(
            np.transpose(np.asarray(a, f), perm)).astype(bf)

    wqT = tb(Wq, (0, 2, 1))
    wkT = tb(Wk, (0, 2, 1))
    wvT = tb(Wv, (0, 2, 1))
    w1T = tb(W1, (0, 2, 1))
    w2T = tb(W2, (0, 2, 1))
    woTh = np.ascontiguousarray(np.transpose(
        np.asarray(Wo, f).reshape(NLAYER, D, H, DH), (0, 3, 2, 1))).astype(bf)

    def col(v):  # [NLAYER, D] -> [NLAYER, P, DB]
        return np.transpose(np.asarray(v, f).reshape(NLAYER, DB, P), (0, 2, 1))

    params = np.zeros((NLAYER, P, NPARAM), f)
    params[:, :, BQ:BQ + DB] = col(inputs["bq"])
    params[:, :, BK:BK + DB] = col(inputs["bk"])
    params[:, :, BV:BV + DB] = col(inputs["bv"])
    params[:, :, BO:BO + DB] = col(inputs["bo"])
    params[:, :, B2S:B2S + DB] = col(inputs["b2"])
    params[:, :, LN1G:LN1G + DB] = col(inputs["ln1_g"])
    params[:, :, LN1B:LN1B + DB] = col(inputs["ln1_b"])
    params[:, :, LN2G:LN2G + DB] = col(inputs["ln2_g"])
    params[:, :, LN2B:LN2B + DB] = col(inputs["ln2_b"])
    params[:, :, B1S:B1S + FB] = np.transpose(
        np.asarray(inputs["b1"], f).reshape(NLAYER, FB, P), (0, 2, 1))

    shared = {
        "cls": np.asarray(inputs["cls_token"], f).reshape(D, 1).astype(bf),
        "wqT": wqT, "wkT": wkT, "wvT": wvT, "woTh": woTh,
        "w1T": w1T, "w2T": w2T, "params": params,
        "bvrow": np.ascontiguousarray(np.asarray(inputs["bv"], f)),
    }
    e1 = np.asarray(inputs["embed1"], f).astype(bf)
    e2 = np.asarray(inputs["embed2"], f).astype(bf)
    in_maps = []
    for b in range(N_CORES):
        m = dict(shared)
        m["e1"] = np.ascontiguousarray(e1[b])
        m["e2"] = np.ascontiguousarray(e2[b])
        in_maps.append(m)
    return in_maps


def _run(inputs, trace=False, **kw):
    from concourse.bass_utils import run_bass_kernel_spmd

    if "nc" not in _CACHE:
        _CACHE["nc"] = _build_nc()
    nc = _CACHE["nc"]
    in_maps = _prep_inputs(inputs)
    res = run_bass_kernel_spmd(nc, in_maps, list(range(N_CORES)), trace=trace,
                               **kw)
    out = np.stack([res.results[b]["out"] for b in range(N_CORES)], axis=0)
    return out.astype(np.float32), res


def kernel(**inputs):
    out, _ = _run(inputs, trace=False)
    return out


# revision 11
# speedup vs baseline: 1.5601x; 1.0510x over previous
"""Trainium2 Bass kernel for a 2-layer cross-attention dense transformer.

Sharding: data-parallel over batch — B=8 batch elements, one per NeuronCore.
Each core runs the full 2-layer transformer on its batch element; weights are
replicated. No collectives.

v2 layout/schedule choices (vs v1 baseline at 931us):
  - All matmul operands in bf16 (fp32 PSUM accumulation). Halves weight DMA,
    enables 2x/4x DVE modes, same 1 cycle/row PE rate as f32r.
  - Scores for a head PAIR go into one 2-bank PSUM tile; ONE exp ACTIVATE
    covers both heads (684 cols), halving ScalarE instruction overheads.
  - exp pool holds 18 pair-tiles so attnV(pair p) never starves exp(p+1);
    scores for pair p+1 are emitted BEFORE attnV(p) to keep PE busy while
    ScalarE catches up.
  - LN affine runs on DVE (tensor ops + two-scalar tensor_scalar), keeping
    ScalarE for exp/gelu only.
  - All weight DMAs for a layer are issued at layer start so W1/W2 land
    during attention.
"""

import numpy as np

# ---------------------------------------------------------------- constants
B, D, L0 = 8, 512, 1024
L = L0 + 1            # 1025 tokens (cls + 1024)
H, DH = 8, 64
DFF = 2048
NLAYER = 2
EPS = 1e-6
SCALE = 1.0 / (DH ** 0.5)

P = 128
DB = D // P           # 4 feature blocks
FB = DFF // P         # 16 dff blocks
LB = 9                # l-tiles over length (8 full + 1 row)

N_CORES = 8

CW = 352                   # padded chunk width
LP = 1026                  # padded length: 3 even chunks of 342, no edges
QC = [(0, 342), (342, 342), (684, 342)]

# params tile slot indices (free-dim column j of the [128, NPARAM] tile)
BQ, BK, BV, BO, B2S = 0, 4, 8, 12, 16
LN1G, LN1B, LN2G, LN2B = 20, 24, 28, 32
B1S = 36
NPARAM = 52

_CACHE = {}


# ---------------------------------------------------------------- bass build
def _build_nc():
    import concourse.bass as bass
    import concourse.bacc as bacc
    import concourse.tile as tile
    from concourse import mybir
    from concourse.masks import make_identity

    f32 = mybir.dt.float32
    f32r = mybir.dt.float32r
    bf16 = mybir.dt.bfloat16
    AO = mybir.AluOpType
    AF = mybir.ActivationFunctionType

    nc = bacc.Bacc("TRN2", target_bir_lowering=False, debug=False)

    # ---- DRAM I/O (per core) ----
    e1 = nc.dram_tensor("e1", [D, L0], bf16, kind="ExternalInput")
    e2 = nc.dram_tensor("e2", [D, L0], bf16, kind="ExternalInput")
    cls_t = nc.dram_tensor("cls", [D, 1], bf16, kind="ExternalInput")
    wqT = nc.dram_tensor("wqT", [NLAYER, D, D], bf16, kind="ExternalInput")
    wkT = nc.dram_tensor("wkT", [NLAYER, D, D], bf16, kind="ExternalInput")
    wvT = nc.dram_tensor("wvT", [NLAYER, D, D], bf16, kind="ExternalInput")
    woTh = nc.dram_tensor("woTh", [NLAYER, DH, H, D], bf16,
                          kind="ExternalInput")
    w1T = nc.dram_tensor("w1T", [NLAYER, D, DFF], bf16, kind="ExternalInput")
    w2T = nc.dram_tensor("w2T", [NLAYER, DFF, D], bf16, kind="ExternalInput")
    params_d = nc.dram_tensor("params", [NLAYER, P, NPARAM], f32,
                              kind="ExternalInput")
    bvrow_d = nc.dram_tensor("bvrow", [NLAYER, D], f32, kind="ExternalInput")
    out_d = nc.dram_tensor("out", [L, D], f32, kind="ExternalOutput")

    with tile.TileContext(nc) as tc:
        with tc.tile_pool(name="persist", bufs=1) as pp, \
             tc.tile_pool(name="xpool", bufs=1) as xp, \
             tc.tile_pool(name="parms", bufs=2) as prm_pool:

            ones_bf = pp.tile([P, P], bf16)
            nc.vector.memset(ones_bf[:], 1.0)
            ones_col = pp.tile([P, 1], bf16)
            nc.vector.tensor_copy(ones_col[:], ones_bf[:, 0:1])
            ones_row = pp.tile([1, P], bf16)
            nc.vector.tensor_copy(ones_row[:], ones_bf[0:1, :])
            ident = pp.tile([P, P], bf16)
            make_identity(nc, ident[:])
            eps_row = pp.tile([1, 1], f32)
            nc.vector.memset(eps_row[:], EPS)

            x = xp.tile([P, DB, LP], bf16, tag="x")
            x2 = xp.tile([P, DB, LP], bf16, tag="x2")
            xmid = xp.tile([P, DB, LP], bf16, tag="xmid")

            for m in range(DB):
                nc.sync.dma_start(x2[:, m, 1:L], e2[m * P:(m + 1) * P, :])
                nc.sync.dma_start(x2[:, m, 0:1], cls_t[m * P:(m + 1) * P, :])
            for m in range(DB):
                nc.sync.dma_start(x[:, m, 1:L], e1[m * P:(m + 1) * P, :])
                nc.sync.dma_start(x[:, m, 0:1], cls_t[m * P:(m + 1) * P, :])
            # zero the single pad token column (keeps everything finite)
            nc.vector.tensor_scalar_mul(x[:, :, L], ones_bf[:, 0:DB], 0.0)
            nc.vector.tensor_scalar_mul(x2[:, :, L], ones_bf[:, 0:DB], 0.0)

            def layernorm(psp, pstag, rows, sqpool, sqtag, src, prms,
                          GSLOT, BSLOT, out_ap):
                """LN over features of src [P, DB, L] -> out_ap (may alias).

                Row stats accumulate across all q-chunks first, the rsqrt
                runs once per site, then the affine applies per chunk on DVE.
                """
                m_full = rows.tile([1, LP], f32, tag="lnm")
                s_full = rows.tile([1, LP], f32, tag="lns")
                for (qo, qw) in QC:
                    sq = sqpool.tile([P, DB, CW], bf16, tag=sqtag)
                    nc.vector.tensor_mul(sq[:, :, :qw], src[:, :, qo:qo + qw],
                                         src[:, :, qo:qo + qw])
                    mp = psp.tile([P, 512], f32, tag=pstag)
                    for kt in range(DB):
                        nc.tensor.matmul(mp[0:1, :qw], ones_col[:, 0:1],
                                         src[:, kt, qo:qo + qw],
                                         start=(kt == 0), stop=(kt == DB - 1))
                    sp = psp.tile([P, 512], f32, tag=pstag)
                    for kt in range(DB):
                        nc.tensor.matmul(sp[0:1, :qw], ones_col[:, 0:1],
                                         sq[:, kt, :qw],
                                         start=(kt == 0), stop=(kt == DB - 1))
                    nc.vector.tensor_scalar_mul(m_full[:, qo:qo + qw],
                                                mp[0:1, :qw], 1.0 / D)
                    nc.vector.tensor_scalar_mul(s_full[:, qo:qo + qw],
                                                sp[0:1, :qw], 1.0 / D)
                # var = E[x^2] - mean^2 ; rstd = exp(-0.5*ln(var+eps))
                msq = rows.tile([1, LP], f32, tag="lnt")
                nc.vector.tensor_mul(msq[:, :], m_full[:, :], m_full[:, :])
                nc.vector.tensor_sub(s_full[:, :], s_full[:, :], msq[:, :])
                nc.scalar.activation(s_full[:, :], s_full[:, :], AF.Ln,
                                     bias=eps_row[:])
                nc.vector.tensor_scalar_mul(s_full[:, :], s_full[:, :], -0.5)
                r_full = rows.tile([1, LP], f32, tag="lnr")
                nc.scalar.activation(r_full[:, :], s_full[:, :], AF.Exp)
                m_bf = rows.tile([1, LP], bf16, tag="lnmb")
                nc.vector.tensor_copy(m_bf[:, :], m_full[:, :])
                r_bf = rows.tile([1, LP], bf16, tag="lnrb")
                nc.vector.tensor_copy(r_bf[:, :], r_full[:, :])
                for (qo, qw) in QC:
                    mb = psp.tile([P, 512], f32, tag=pstag)
                    nc.tensor.matmul(mb[:, :qw], ones_row[0:1, :],
                                     m_bf[0:1, qo:qo + qw],
                                     start=True, stop=True)
                    rb = psp.tile([P, 512], f32, tag=pstag)
                    nc.tensor.matmul(rb[:, :qw], ones_row[0:1, :],
                                     r_bf[0:1, qo:qo + qw],
                                     start=True, stop=True)
                    mbb = sqpool.tile([P, CW], bf16, tag=sqtag + "mb")
                    nc.vector.tensor_copy(mbb[:, :qw], mb[:, :qw])
                    rbb = sqpool.tile([P, CW], bf16, tag=sqtag + "rb")
                    nc.vector.tensor_copy(rbb[:, :qw], rb[:, :qw])
                    sq = sqpool.tile([P, DB, CW], bf16, tag=sqtag)
                    for m in range(DB):
                        nc.vector.tensor_sub(sq[:, m, :qw],
                                             src[:, m, qo:qo + qw],
                                             mbb[:, :qw])
                        nc.vector.tensor_mul(sq[:, m, :qw], sq[:, m, :qw],
                                             rbb[:, :qw])
                        nc.vector.tensor_scalar(
                            out=out_ap[:, m, qo:qo + qw], in0=sq[:, m, :qw],
                            scalar1=prms[:, GSLOT + m:GSLOT + m + 1],
                            scalar2=prms[:, BSLOT + m:BSLOT + m + 1],
                            op0=AO.mult, op1=AO.add)

            for l in range(NLAYER):
                prms = prm_pool.tile([P, NPARAM], f32, tag="prms")
                nc.sync.dma_start(prms[:], params_d[l, :, :])

                with tc.tile_pool(name=f"wA{l}", bufs=1) as wp, \
                     tc.tile_pool(name=f"woA{l}", bufs=1) as wop, \
                     tc.tile_pool(name=f"kvA{l}", bufs=1) as ap1, \
                     tc.tile_pool(name=f"exA{l}", bufs=18) as exl, \
                     tc.tile_pool(name=f"sbA{l}", bufs=1) as ap3, \
                     tc.tile_pool(name=f"fwB{l}", bufs=1) as fwp, \
                     tc.tile_pool(name=f"rwA{l}", bufs=1) as rows, \
                     tc.tile_pool(name=f"raA{l}", bufs=4) as rrows, \
                     tc.tile_pool(name=f"psA{l}", bufs=2, space="PSUM") as psA, \
                     tc.tile_pool(name=f"psC{l}", bufs=2, space="PSUM") as psC, \
                     tc.tile_pool(name=f"psW{l}", bufs=2, space="PSUM") as psW:

                    # ---- issue ALL layer weight DMAs up front ----
                    wk_sb = wp.tile([P, DB, D], bf16, tag="wk")
                    nc.sync.dma_start(
                        wk_sb[:],
                        wkT[l, :, :].rearrange("(b p) n -> p b n", p=P))
                    wv_sb = wp.tile([P, DB, D], bf16, tag="wv")
                    nc.sync.dma_start(
                        wv_sb[:],
                        wvT[l, :, :].rearrange("(b p) n -> p b n", p=P))
                    wq_sb = wp.tile([P, DB, D], bf16, tag="wq")
                    nc.sync.dma_start(
                        wq_sb[:],
                        wqT[l, :, :].rearrange("(b p) n -> p b n", p=P))
                    wo_sb = wop.tile([DH, H, D], bf16, tag="wo")
                    nc.sync.dma_start(wo_sb[:], woTh[l, :, :, :])
                    w1_sb = fwp.tile([P, DB, DFF], bf16, tag="w1")
                    nc.sync.dma_start(
                        w1_sb[:],
                        w1T[l, :, :].rearrange("(b p) n -> p b n", p=P))
                    w2_sb = fwp.tile([P, FB, D], bf16, tag="w2")
                    nc.sync.dma_start(
                        w2_sb[:],
                        w2T[l, :, :].rearrange("(b p) n -> p b n", p=P))
                    bvb = ap1.tile([P, D], f32, tag="bvb")
                    nc.sync.dma_start(
                        bvb[:],
                        bass.AP(tensor=bvrow_d, offset=l * D,
                                ap=[[0, P], [1, D]]))

                    # ---- K projection (full L); softmax scale folded in ----
                    K_fm = ap1.tile([P, DB, LP], f32r, tag="K")
                    for m in range(DB):
                        for (o, w) in QC:
                            kp = psA.tile([P, 2, 512], f32, tag="psA")
                            for kt in range(DB):
                                nc.tensor.matmul(
                                    kp[:, 0, :w],
                                    wk_sb[:, kt, m * P:(m + 1) * P],
                                    x2[:, kt, o:o + w],
                                    start=(kt == 0), stop=(kt == DB - 1))
                            nc.vector.tensor_scalar(
                                out=K_fm[:, m, o:o + w], in0=kp[:, 0, :w],
                                scalar1=prms[:, BK + m:BK + m + 1],
                                scalar2=SCALE, op0=AO.add, op1=AO.mult)

                    # ---- V projection (token-major, ones column at DH) ----
                    V_tm = ap1.tile([P, LB, H, DH + 1], bf16, tag="V")
                    nc.vector.tensor_copy(
                        V_tm[:, :, :, DH],
                        ones_bf[:, 0:LB * H].rearrange("p (a b) -> p a b",
                                                       a=LB))
                    for mt in range(LB):
                        nrow = P if mt < LB - 1 else L - (LB - 1) * P
                        vp = psA.tile([P, 2, 512], f32, tag="psA")
                        for kt in range(DB):
                            nc.tensor.matmul(
                                vp[:nrow, 0, :D],
                                x2[:, kt, mt * P:mt * P + nrow],
                                wv_sb[:, kt, :],
                                start=(kt == 0), stop=(kt == DB - 1))
                        nc.vector.tensor_tensor(
                            out=V_tm[:nrow, mt, :, 0:DH],
                            in0=vp[:nrow, 0, :D].rearrange("p (h c) -> p h c",
                                                           h=H),
                            in1=bvb[:nrow, :].rearrange("p (h c) -> p h c",
                                                        h=H),
                            op=AO.add)

                    # ---- per q-chunk attention, software-pipelined ----
                    # Emission order interleaves at kt granularity so the PE
                    # always has independent attnV/outproj work queued while
                    # ScalarE chews through the exps of the newest scores.
                    def emit_outproj(qo, qw, ctx_sb):
                        for m in range(DB):
                            op_ = psW.tile([P, 512], f32, tag="psW")
                            for h in range(H):
                                nc.tensor.matmul(
                                    op_[:, :qw],
                                    wo_sb[:, h, m * P:(m + 1) * P],
                                    ctx_sb[:, h, :qw],
                                    start=(h == 0), stop=(h == H - 1))
                            nc.vector.scalar_tensor_tensor(
                                out=xmid[:, m, qo:qo + qw], in0=op_[:, :qw],
                                scalar=prms[:, BO + m:BO + m + 1],
                                in1=x[:, m, qo:qo + qw],
                                op0=AO.add, op1=AO.add)
                            yield

                    def scores_kt(hp, kt, qw, Q_fm):
                        """Scores matmuls + one paired exp for (pair, kt)."""
                        h0 = 2 * hp
                        nrow = P if kt < LB - 1 else L - (LB - 1) * P
                        sp = psA.tile([P, 2, 512], f32, tag="psA")
                        for j in range(2):
                            b = ((h0 + j) % 2) * DH
                            bl = (h0 + j) // 2
                            nc.tensor.matmul(
                                sp[:nrow, j, :qw],
                                K_fm[b:b + DH, bl, kt * P:kt * P + nrow],
                                Q_fm[b:b + DH, bl, :qw],
                                start=True, stop=True)
                        et = exl.tile([P, 2, CW], bf16, tag="exp")
                        nc.scalar.activation(et[:nrow, :, :qw],
                                             sp[:nrow, :, :qw], AF.Exp)
                        return et

                    def normalize(h, cp, qw, ctx_sb):
                        drow = rrows.tile([1, CW], f32, tag="row")
                        nc.vector.tensor_copy(drow[:, :qw],
                                              cp[DH:DH + 1, :qw])
                        rrow = rrows.tile([1, CW], f32, tag="row")
                        nc.vector.reciprocal_approx_fast(
                            rrow[:, :qw], drow[:, :qw])
                        rrowr = rrows.tile([1, CW], bf16, tag="rowb")
                        nc.vector.tensor_copy(rrowr[:, :qw], rrow[:, :qw])
                        rb = psW.tile([P, 512], f32, tag="psW")
                        nc.tensor.matmul(rb[:DH, :qw], ones_row[0:1, 0:DH],
                                         rrowr[0:1, :qw],
                                         start=True, stop=True)
                        nc.vector.tensor_copy(ctx_sb[:, h, :qw],
                                              cp[:DH, :qw])
                        nc.vector.tensor_tensor(
                            out=ctx_sb[:, h, :qw], in0=ctx_sb[:, h, :qw],
                            in1=rb[:DH, :qw], op=AO.mult)

                    carry = None   # outproj generator of previous chunk
                    for ci, (qo, qw) in enumerate(QC):
                        Q_fm = ap3.tile([P, DB, CW], f32r, tag="Q")
                        for m in range(DB):
                            qp = psW.tile([P, 512], f32, tag="psW")
                            for kt in range(DB):
                                nc.tensor.matmul(
                                    qp[:, :qw],
                                    wq_sb[:, kt, m * P:(m + 1) * P],
                                    x[:, kt, qo:qo + qw],
                                    start=(kt == 0), stop=(kt == DB - 1))
                            nc.vector.tensor_scalar_add(
                                Q_fm[:, m, :qw], qp[:, :qw],
                                prms[:, BQ + m:BQ + m + 1])

                        ctx_sb = ap3.tile([DH, H, CW], bf16, tag="ctx")

                        # lead-in: scores(pair 0) interleaved with the
                        # previous chunk's output projection
                        ets_pend = []
                        for kt in range(LB):
                            ets_pend.append(scores_kt(0, kt, qw, Q_fm))
                            if carry is not None and kt % 2 == 0:
                                next(carry, None)
                        if carry is not None:
                            for _ in carry:
                                pass

                        for hp in range(H // 2):
                            nxt = hp + 1 < H // 2
                            h0 = 2 * hp
                            cp0 = psC.tile([DH + 1, 512], f32, tag="psC")
                            cp1 = psC.tile([DH + 1, 512], f32, tag="psC")
                            ets_new = []
                            for kt in range(LB):
                                nrow = (P if kt < LB - 1
                                        else L - (LB - 1) * P)
                                if nxt:
                                    ets_new.append(
                                        scores_kt(hp + 1, kt, qw, Q_fm))
                                nc.tensor.matmul(
                                    cp0[:, :qw], V_tm[:nrow, kt, h0, :],
                                    ets_pend[kt][:nrow, 0, :qw],
                                    start=(kt == 0), stop=(kt == LB - 1))
                                nc.tensor.matmul(
                                    cp1[:, :qw], V_tm[:nrow, kt, h0 + 1, :],
                                    ets_pend[kt][:nrow, 1, :qw],
                                    start=(kt == 0), stop=(kt == LB - 1))
                            normalize(h0, cp0, qw, ctx_sb)
                            normalize(h0 + 1, cp1, qw, ctx_sb)
                            ets_pend = ets_new

                        carry = emit_outproj(qo, qw, ctx_sb)

                    # drain the final chunk's output projection
                    for _ in carry:
                        pass

                    # ---- LN1 (in place on xmid) ----
                    layernorm(psW, "psW", rows, ap3, "sq", xmid, prms,
                              LN1G, LN1B, xmid)

                    # =================== PHASE B: FFN ===================
                    with tc.tile_pool(name=f"hB{l}", bufs=1) as fhp:
                        for (qo, qw) in QC:
                            h_sb = fhp.tile([P, FB, CW], bf16, tag="h")
                            for mf in range(FB):
                                hp_ = psA.tile([P, 2, 512], f32, tag="psA")
                                for kt in range(DB):
                                    nc.tensor.matmul(
                                        hp_[:, 0, :qw],
                                        w1_sb[:, kt, mf * P:(mf + 1) * P],
                                        xmid[:, kt, qo:qo + qw],
                                        start=(kt == 0), stop=(kt == DB - 1))
                                nc.scalar.activation(
                                    h_sb[:, mf, :qw], hp_[:, 0, :qw], AF.Gelu,
                                    bias=prms[:, B1S + mf:B1S + mf + 1])
                            for m in range(DB):
                                fp = psW.tile([P, 512], f32, tag="psW")
                                for kt in range(FB):
                                    nc.tensor.matmul(
                                        fp[:, :qw],
                                        w2_sb[:, kt, m * P:(m + 1) * P],
                                        h_sb[:, kt, :qw],
                                        start=(kt == 0), stop=(kt == FB - 1))
                                nc.vector.scalar_tensor_tensor(
                                    out=x[:, m, qo:qo + qw], in0=fp[:, :qw],
                                    scalar=prms[:, B2S + m:B2S + m + 1],
                                    in1=xmid[:, m, qo:qo + qw],
                                    op0=AO.add, op1=AO.add)

                        # ---- LN2 (in place on x) ----
                        layernorm(psW, "psW", rows, ap3, "sq", x, prms,
                                  LN2G, LN2B, x)

            # =================== transpose x -> out ===================
            # 4 feature-block transposes land in one PSUM tile, then one
            # [128, 512] cast-copy and one contiguous row-block DMA.
            with tc.tile_pool(name="psT", bufs=3, space="PSUM") as psT, \
                 tc.tile_pool(name="sbT", bufs=3) as sbT:
                for mt in range(LB):
                    nrow = P if mt < LB - 1 else L - (LB - 1) * P
                    tp = psT.tile([P, DB, P], bf16, tag="psT")
                    for m in range(DB):
                        nc.tensor.transpose(
                            tp[:nrow, m, :],
                            x[:, m, mt * P:mt * P + nrow],
                            ident[:])
                    ts = sbT.tile([P, DB * P], f32, tag="sbT")
                    nc.vector.tensor_copy(
                        ts[:nrow, :],
                        tp[:nrow, :, :].rearrange("p a b -> p (a b)"))
                    nc.sync.dma_start(
                        out_d[mt * P:mt * P + nrow, :], ts[:nrow, :])

    nc.compile()
    return nc


# ---------------------------------------------------------------- host side
def _prep_inputs(inputs):
    import ml_dtypes
    f = np.float32
    bf = ml_dtypes.bfloat16
    Wq, Wk, Wv, Wo = inputs["Wq"], inputs["Wk"], inputs["Wv"], inputs["Wo"]
    W1, W2 = inputs["W1"], inputs["W2"]

    def tb(a, perm):
        return np.ascontiguousarray(
            np.transpose(np.asarray(a, f), perm)).astype(bf)

    wqT = tb(Wq, (0, 2, 1))
    wkT = tb(Wk, (0, 2, 1))
    wvT = tb(Wv, (0, 2, 1))
    w1T = tb(W1, (0, 2, 1))
    w2T = tb(W2, (0, 2, 1))
    woTh = np.ascontiguousarray(np.transpose(
        np.asarray(Wo, f).reshape(NLAYER, D, H, DH), (0, 3, 2, 1))).astype(bf)

    def col(v):  # [NLAYER, D] -> [NLAYER, P, DB]
        return np.transpose(np.asarray(v, f).reshape(NLAYER, DB, P), (0, 2, 1))

    params = np.zeros((NLAYER, P, NPARAM), f)
    params[:, :, BQ:BQ + DB] = col(inputs["bq"])
    params[:, :, BK:BK + DB] = col(inputs["bk"])
    params[:, :, BV:BV + DB] = col(inputs["bv"])
    params[:, :, BO:BO + DB] = col(inputs["bo"])
    params[:, :, B2S:B2S + DB] = col(inputs["b2"])
    params[:, :, LN1G:LN1G + DB] = col(inputs["ln1_g"])
    params[:, :, LN1B:LN1B + DB] = col(inputs["ln1_b"])
    params[:, :, LN2G:LN2G + DB] = col(inputs["ln2_g"])
    params[:, :, LN2B:LN2B + DB] = col(inputs["ln2_b"])
    params[:, :, B1S:B1S + FB] = np.transpose(
        np.asarray(inputs["b1"], f).reshape(NLAYER, FB, P), (0, 2, 1))

    shared = {
        "cls": np.asarray(inputs["cls_token"], f).reshape(D, 1).astype(bf),
        "wqT": wqT, "wkT": wkT, "wvT": wvT, "woTh": woTh,
        "w1T": w1T, "w2T": w2T, "params": params,
        "bvrow": np.ascontiguousarray(np.asarray(inputs["bv"], f)),
    }
    e1 = np.asarray(inputs["embed1"], f).astype(bf)
    e2 = np.asarray(inputs["embed2"], f).astype(bf)
    in_maps = []
    for b in range(N_CORES):
        m = dict(shared)
        m["e1"] = np.ascontiguousarray(e1[b])
        m["e2"] = np.ascontiguousarray(e2[b])
        in_maps.append(m)
    return in_maps


def _run(inputs, trace=False, **kw):
    from concourse.bass_utils import run_bass_kernel_spmd

    if "nc" not in _CACHE:
        _CACHE["nc"] = _build_nc()
    nc = _CACHE["nc"]
    in_maps = _prep_inputs(inputs)
    res = run_bass_kernel_spmd(nc, in_maps, list(range(N_CORES)), trace=trace,
                               **kw)
    out = np.stack([res.results[b]["out"] for b in range(N_CORES)], axis=0)
    return out.astype(np.float32), res


def kernel(**inputs):
    out, _ = _run(inputs, trace=False)
    return out


# revision 16
# speedup vs baseline: 1.5636x; 1.0023x over previous
"""Trainium2 Bass kernel for a 2-layer cross-attention dense transformer.

Sharding: data-parallel over batch — B=8 batch elements, one per NeuronCore.
Each core runs the full 2-layer transformer on its batch element; weights are
replicated. No collectives.

v2 layout/schedule choices (vs v1 baseline at 931us):
  - All matmul operands in bf16 (fp32 PSUM accumulation). Halves weight DMA,
    enables 2x/4x DVE modes, same 1 cycle/row PE rate as f32r.
  - Scores for a head PAIR go into one 2-bank PSUM tile; ONE exp ACTIVATE
    covers both heads (684 cols), halving ScalarE instruction overheads.
  - exp pool holds 18 pair-tiles so attnV(pair p) never starves exp(p+1);
    scores for pair p+1 are emitted BEFORE attnV(p) to keep PE busy while
    ScalarE catches up.
  - LN affine runs on DVE (tensor ops + two-scalar tensor_scalar), keeping
    ScalarE for exp/gelu only.
  - All weight DMAs for a layer are issued at layer start so W1/W2 land
    during attention.
"""

import numpy as np

# ---------------------------------------------------------------- constants
B, D, L0 = 8, 512, 1024
L = L0 + 1            # 1025 tokens (cls + 1024)
H, DH = 8, 64
DFF = 2048
NLAYER = 2
EPS = 1e-6
SCALE = 1.0 / (DH ** 0.5)

P = 128
DB = D // P           # 4 feature blocks
FB = DFF // P         # 16 dff blocks
LB = 9                # l-tiles over length (8 full + 1 row)

N_CORES = 8

CW = 352                   # padded chunk width
LP = 1026                  # padded length: 3 even chunks of 342, no edges
QC = [(0, 342), (342, 342), (684, 342)]

# params tile slot indices (free-dim column j of the [128, NPARAM] tile)
BQ, BK, BV, BO, B2S = 0, 4, 8, 12, 16
LN1G, LN1B, LN2G, LN2B = 20, 24, 28, 32
B1S = 36
NPARAM = 52

_CACHE = {}


# ---------------------------------------------------------------- bass build
def _build_nc():
    import concourse.bass as bass
    import concourse.bacc as bacc
    import concourse.tile as tile
    from concourse import mybir
    from concourse.masks import make_identity

    f32 = mybir.dt.float32
    f32r = mybir.dt.float32r
    bf16 = mybir.dt.bfloat16
    AO = mybir.AluOpType
    AF = mybir.ActivationFunctionType

    nc = bacc.Bacc("TRN2", target_bir_lowering=False, debug=False)

    # ---- DRAM I/O (per core) ----
    e1 = nc.dram_tensor("e1", [D, L0], bf16, kind="ExternalInput")
    e2 = nc.dram_tensor("e2", [D, L0], bf16, kind="ExternalInput")
    cls_t = nc.dram_tensor("cls", [D, 1], bf16, kind="ExternalInput")
    wqT = nc.dram_tensor("wqT", [NLAYER, D, D], bf16, kind="ExternalInput")
    wkT = nc.dram_tensor("wkT", [NLAYER, D, D], bf16, kind="ExternalInput")
    wvT = nc.dram_tensor("wvT", [NLAYER, D, D], bf16, kind="ExternalInput")
    woTh = nc.dram_tensor("woTh", [NLAYER, DH, H, D], bf16,
                          kind="ExternalInput")
    w1T = nc.dram_tensor("w1T", [NLAYER, D, DFF], bf16, kind="ExternalInput")
    w2T = nc.dram_tensor("w2T", [NLAYER, DFF, D], bf16, kind="ExternalInput")
    params_d = nc.dram_tensor("params", [NLAYER, P, NPARAM], f32,
                              kind="ExternalInput")
    bvrow_d = nc.dram_tensor("bvrow", [NLAYER, D], f32, kind="ExternalInput")
    out_d = nc.dram_tensor("out", [L, D], f32, kind="ExternalOutput")

    with tile.TileContext(nc) as tc:
        with tc.tile_pool(name="persist", bufs=1) as pp, \
             tc.tile_pool(name="xpool", bufs=1) as xp, \
             tc.tile_pool(name="parms", bufs=2) as prm_pool:

            ones_bf = pp.tile([P, P], bf16)
            nc.vector.memset(ones_bf[:], 1.0)
            ones_col = pp.tile([P, 1], bf16)
            nc.vector.tensor_copy(ones_col[:], ones_bf[:, 0:1])
            ones_row = pp.tile([1, P], bf16)
            nc.vector.tensor_copy(ones_row[:], ones_bf[0:1, :])
            ident = pp.tile([P, P], bf16)
            make_identity(nc, ident[:])
            eps_row = pp.tile([1, 1], f32)
            nc.vector.memset(eps_row[:], EPS)

            x = xp.tile([P, DB, LP], bf16, tag="x")
            x2 = xp.tile([P, DB, LP], bf16, tag="x2")
            xmid = xp.tile([P, DB, LP], bf16, tag="xmid")
            wk1_pf = xp.tile([P, DB, D], bf16, tag="wk1pf")

            # split big loads across two queue engines for descriptor rate
            for m in range(DB):
                eng = nc.sync if m % 2 == 0 else nc.gpsimd
                eng.dma_start(x2[:, m, 1:L], e2[m * P:(m + 1) * P, :])
                eng.dma_start(x2[:, m, 0:1], cls_t[m * P:(m + 1) * P, :])
            for m in range(DB):
                eng = nc.sync if m % 2 == 0 else nc.gpsimd
                eng.dma_start(x[:, m, 1:L], e1[m * P:(m + 1) * P, :])
                eng.dma_start(x[:, m, 0:1], cls_t[m * P:(m + 1) * P, :])
            # zero the single pad token column (keeps everything finite)
            nc.vector.tensor_scalar_mul(x[:, :, L], ones_bf[:, 0:DB], 0.0)
            nc.vector.tensor_scalar_mul(x2[:, :, L], ones_bf[:, 0:DB], 0.0)

            def layernorm(psp, pstag, rows, sqpool, sqtag, src, prms,
                          GSLOT, BSLOT, out_ap):
                """LN over features of src [P, DB, L] -> out_ap (may alias).

                Row stats accumulate across all q-chunks first, the rsqrt
                runs once per site, then the affine applies per chunk on DVE.
                """
                m_full = rows.tile([1, LP], f32, tag="lnm")
                s_full = rows.tile([1, LP], f32, tag="lns")
                for (qo, qw) in QC:
                    sq = sqpool.tile([P, DB, CW], bf16, tag=sqtag)
                    nc.vector.tensor_mul(sq[:, :, :qw], src[:, :, qo:qo + qw],
                                         src[:, :, qo:qo + qw])
                    mp = psp.tile([P, 512], f32, tag=pstag)
                    for kt in range(DB):
                        nc.tensor.matmul(mp[0:1, :qw], ones_col[:, 0:1],
                                         src[:, kt, qo:qo + qw],
                                         start=(kt == 0), stop=(kt == DB - 1))
                    sp = psp.tile([P, 512], f32, tag=pstag)
                    for kt in range(DB):
                        nc.tensor.matmul(sp[0:1, :qw], ones_col[:, 0:1],
                                         sq[:, kt, :qw],
                                         start=(kt == 0), stop=(kt == DB - 1))
                    nc.vector.tensor_scalar_mul(m_full[:, qo:qo + qw],
                                                mp[0:1, :qw], 1.0 / D)
                    nc.vector.tensor_scalar_mul(s_full[:, qo:qo + qw],
                                                sp[0:1, :qw], 1.0 / D)
                # var = E[x^2] - mean^2 ; rstd = exp(-0.5*ln(var+eps))
                msq = rows.tile([1, LP], f32, tag="lnt")
                nc.vector.tensor_mul(msq[:, :], m_full[:, :], m_full[:, :])
                nc.vector.tensor_sub(s_full[:, :], s_full[:, :], msq[:, :])
                nc.scalar.activation(s_full[:, :], s_full[:, :], AF.Ln,
                                     bias=eps_row[:])
                nc.vector.tensor_scalar_mul(s_full[:, :], s_full[:, :], -0.5)
                r_full = rows.tile([1, LP], f32, tag="lnr")
                nc.scalar.activation(r_full[:, :], s_full[:, :], AF.Exp)
                m_bf = rows.tile([1, LP], bf16, tag="lnmb")
                nc.vector.tensor_copy(m_bf[:, :], m_full[:, :])
                r_bf = rows.tile([1, LP], bf16, tag="lnrb")
                nc.vector.tensor_copy(r_bf[:, :], r_full[:, :])
                for (qo, qw) in QC:
                    mb = psp.tile([P, 512], f32, tag=pstag)
                    nc.tensor.matmul(mb[:, :qw], ones_row[0:1, :],
                                     m_bf[0:1, qo:qo + qw],
                                     start=True, stop=True)
                    rb = psp.tile([P, 512], f32, tag=pstag)
                    nc.tensor.matmul(rb[:, :qw], ones_row[0:1, :],
                                     r_bf[0:1, qo:qo + qw],
                                     start=True, stop=True)
                    mbb = sqpool.tile([P, CW], bf16, tag=sqtag + "mb")
                    nc.vector.tensor_copy(mbb[:, :qw], mb[:, :qw])
                    rbb = sqpool.tile([P, CW], bf16, tag=sqtag + "rb")
                    nc.vector.tensor_copy(rbb[:, :qw], rb[:, :qw])
                    sq = sqpool.tile([P, DB, CW], bf16, tag=sqtag)
                    for m in range(DB):
                        nc.vector.tensor_sub(sq[:, m, :qw],
                                             src[:, m, qo:qo + qw],
                                             mbb[:, :qw])
                        nc.vector.tensor_mul(sq[:, m, :qw], sq[:, m, :qw],
                                             rbb[:, :qw])
                        nc.vector.tensor_scalar(
                            out=out_ap[:, m, qo:qo + qw], in0=sq[:, m, :qw],
                            scalar1=prms[:, GSLOT + m:GSLOT + m + 1],
                            scalar2=prms[:, BSLOT + m:BSLOT + m + 1],
                            op0=AO.mult, op1=AO.add)

            for l in range(NLAYER):
                prms = prm_pool.tile([P, NPARAM], f32, tag="prms")
                nc.sync.dma_start(prms[:], params_d[l, :, :])

                with tc.tile_pool(name=f"wA{l}", bufs=1) as wp, \
                     tc.tile_pool(name=f"woA{l}", bufs=1) as wop, \
                     tc.tile_pool(name=f"kvA{l}", bufs=1) as ap1, \
                     tc.tile_pool(name=f"exA{l}", bufs=16) as exl, \
                     tc.tile_pool(name=f"sbA{l}", bufs=1) as ap3, \
                     tc.tile_pool(name=f"fwB{l}", bufs=1) as fwp, \
                     tc.tile_pool(name=f"rwA{l}", bufs=1) as rows, \
                     tc.tile_pool(name=f"raA{l}", bufs=4) as rrows, \
                     tc.tile_pool(name=f"psA{l}", bufs=2, space="PSUM") as psA, \
                     tc.tile_pool(name=f"psC{l}", bufs=2, space="PSUM") as psC, \
                     tc.tile_pool(name=f"psW{l}", bufs=2, space="PSUM") as psW:

                    # ---- issue ALL layer weight DMAs up front ----
                    if l == 0:
                        wk_sb = wp.tile([P, DB, D], bf16, tag="wk")
                        nc.sync.dma_start(
                            wk_sb[:, 0:2, :],
                            wkT[l, 0:2 * P, :].rearrange("(b p) n -> p b n",
                                                         p=P))
                        nc.gpsimd.dma_start(
                            wk_sb[:, 2:4, :],
                            wkT[l, 2 * P:4 * P, :].rearrange(
                                "(b p) n -> p b n", p=P))
                    else:
                        wk_sb = wk1_pf   # prefetched during layer 0
                    wv_sb = wp.tile([P, DB, D], bf16, tag="wv")
                    nc.sync.dma_start(
                        wv_sb[:],
                        wvT[l, :, :].rearrange("(b p) n -> p b n", p=P))
                    wq_sb = wp.tile([P, DB, D], bf16, tag="wq")
                    nc.sync.dma_start(
                        wq_sb[:],
                        wqT[l, :, :].rearrange("(b p) n -> p b n", p=P))
                    wo_sb = wop.tile([DH, H, D], bf16, tag="wo")
                    nc.sync.dma_start(wo_sb[:], woTh[l, :, :, :])
                    w1_sb = fwp.tile([P, DB, DFF], bf16, tag="w1")
                    nc.sync.dma_start(
                        w1_sb[:],
                        w1T[l, :, :].rearrange("(b p) n -> p b n", p=P))
                    w2_sb = fwp.tile([P, FB, D], bf16, tag="w2")
                    nc.sync.dma_start(
                        w2_sb[:],
                        w2T[l, :, :].rearrange("(b p) n -> p b n", p=P))
                    bvb = ap1.tile([P, D], f32, tag="bvb")
                    nc.sync.dma_start(
                        bvb[:],
                        bass.AP(tensor=bvrow_d, offset=l * D,
                                ap=[[0, P], [1, D]]))
                    if l == 0:
                        # prefetch layer 1's Wk so its K-proj never stalls
                        nc.sync.dma_start(
                            wk1_pf[:, 0:2, :],
                            wkT[1, 0:2 * P, :].rearrange("(b p) n -> p b n",
                                                         p=P))
                        nc.gpsimd.dma_start(
                            wk1_pf[:, 2:4, :],
                            wkT[1, 2 * P:4 * P, :].rearrange(
                                "(b p) n -> p b n", p=P))

                    # ---- K projection (full L); softmax scale folded in ----
                    K_fm = ap1.tile([P, DB, LP], f32r, tag="K")
                    for m in range(DB):
                        for (o, w) in QC:
                            kp = psA.tile([P, 2, 512], f32, tag="psA")
                            for kt in range(DB):
                                nc.tensor.matmul(
                                    kp[:, 0, :w],
                                    wk_sb[:, kt, m * P:(m + 1) * P],
                                    x2[:, kt, o:o + w],
                                    start=(kt == 0), stop=(kt == DB - 1))
                            nc.vector.tensor_scalar(
                                out=K_fm[:, m, o:o + w], in0=kp[:, 0, :w],
                                scalar1=prms[:, BK + m:BK + m + 1],
                                scalar2=SCALE, op0=AO.add, op1=AO.mult)

                    # ---- V projection (token-major, ones column at DH) ----
                    V_tm = ap1.tile([P, LB, H, DH + 1], bf16, tag="V")
                    nc.vector.tensor_copy(
                        V_tm[:, :, :, DH],
                        ones_bf[:, 0:LB * H].rearrange("p (a b) -> p a b",
                                                       a=LB))
                    for mt in range(LB):
                        nrow = P if mt < LB - 1 else L - (LB - 1) * P
                        vp = psA.tile([P, 2, 512], f32, tag="psA")
                        for kt in range(DB):
                            nc.tensor.matmul(
                                vp[:nrow, 0, :D],
                                x2[:, kt, mt * P:mt * P + nrow],
                                wv_sb[:, kt, :],
                                start=(kt == 0), stop=(kt == DB - 1))
                        nc.vector.tensor_tensor(
                            out=V_tm[:nrow, mt, :, 0:DH],
                            in0=vp[:nrow, 0, :D].rearrange("p (h c) -> p h c",
                                                           h=H),
                            in1=bvb[:nrow, :].rearrange("p (h c) -> p h c",
                                                        h=H),
                            op=AO.add)

                    # ---- per q-chunk attention, software-pipelined ----
                    # Emission order interleaves at kt granularity so the PE
                    # always has independent attnV/outproj work queued while
                    # ScalarE chews through the exps of the newest scores.
                    def emit_outproj(qo, qw, ctx_sb):
                        for m in range(DB):
                            op_ = psW.tile([P, 512], f32, tag="psW")
                            for h in range(H):
                                nc.tensor.matmul(
                                    op_[:, :qw],
                                    wo_sb[:, h, m * P:(m + 1) * P],
                                    ctx_sb[:, h, :qw],
                                    start=(h == 0), stop=(h == H - 1))
                            nc.vector.scalar_tensor_tensor(
                                out=xmid[:, m, qo:qo + qw], in0=op_[:, :qw],
                                scalar=prms[:, BO + m:BO + m + 1],
                                in1=x[:, m, qo:qo + qw],
                                op0=AO.add, op1=AO.add)
                            yield

                    def scores_kt(hp, kt, qw, Q_fm):
                        """Scores matmuls + one paired exp for (pair, kt)."""
                        h0 = 2 * hp
                        nrow = P if kt < LB - 1 else L - (LB - 1) * P
                        sp = psA.tile([P, 2, 512], f32, tag="psA")
                        for j in range(2):
                            b = ((h0 + j) % 2) * DH
                            bl = (h0 + j) // 2
                            nc.tensor.matmul(
                                sp[:nrow, j, :qw],
                                K_fm[b:b + DH, bl, kt * P:kt * P + nrow],
                                Q_fm[b:b + DH, bl, :qw],
                                start=True, stop=True)
                        et = exl.tile([P, 2, CW], bf16, tag="exp")
                        nc.scalar.activation(et[:nrow, :, :qw],
                                             sp[:nrow, :, :qw], AF.Exp)
                        return et

                    def normalize(h, cp, qw, ctx_sb):
                        drow = rrows.tile([1, CW], f32, tag="row")
                        nc.vector.tensor_copy(drow[:, :qw],
                                              cp[DH:DH + 1, :qw])
                        rrow = rrows.tile([1, CW], f32, tag="row")
                        nc.vector.reciprocal_approx_fast(
                            rrow[:, :qw], drow[:, :qw])
                        rrowr = rrows.tile([1, CW], bf16, tag="rowb")
                        nc.vector.tensor_copy(rrowr[:, :qw], rrow[:, :qw])
                        rb = psW.tile([P, 512], f32, tag="psW")
                        nc.tensor.matmul(rb[:DH, :qw], ones_row[0:1, 0:DH],
                                         rrowr[0:1, :qw],
                                         start=True, stop=True)
                        nc.vector.tensor_copy(ctx_sb[:, h, :qw],
                                              cp[:DH, :qw])
                        nc.vector.tensor_tensor(
                            out=ctx_sb[:, h, :qw], in0=ctx_sb[:, h, :qw],
                            in1=rb[:DH, :qw], op=AO.mult)

                    carry = None   # outproj generator of previous chunk
                    for ci, (qo, qw) in enumerate(QC):
                        Q_fm = ap3.tile([P, DB, CW], f32r, tag="Q")
                        for m in range(DB):
                            qp = psW.tile([P, 512], f32, tag="psW")
                            for kt in range(DB):
                                nc.tensor.matmul(
                                    qp[:, :qw],
                                    wq_sb[:, kt, m * P:(m + 1) * P],
                                    x[:, kt, qo:qo + qw],
                                    start=(kt == 0), stop=(kt == DB - 1))
                            nc.vector.tensor_scalar_add(
                                Q_fm[:, m, :qw], qp[:, :qw],
                                prms[:, BQ + m:BQ + m + 1])

                        ctx_sb = ap3.tile([DH, H, CW], bf16, tag="ctx")

                        # lead-in: scores(pair 0) interleaved with the
                        # previous chunk's output projection
                        ets_pend = []
                        for kt in range(LB):
                            ets_pend.append(scores_kt(0, kt, qw, Q_fm))
                            if carry is not None and kt % 2 == 0:
                                next(carry, None)
                        if carry is not None:
                            for _ in carry:
                                pass

                        for hp in range(H // 2):
                            nxt = hp + 1 < H // 2
                            h0 = 2 * hp
                            cp0 = psC.tile([DH + 1, 512], f32, tag="psC")
                            cp1 = psC.tile([DH + 1, 512], f32, tag="psC")
                            ets_new = []
                            for kt in range(LB):
                                nrow = (P if kt < LB - 1
                                        else L - (LB - 1) * P)
                                if nxt:
                                    ets_new.append(
                                        scores_kt(hp + 1, kt, qw, Q_fm))
                                nc.tensor.matmul(
                                    cp0[:, :qw], V_tm[:nrow, kt, h0, :],
                                    ets_pend[kt][:nrow, 0, :qw],
                                    start=(kt == 0), stop=(kt == LB - 1))
                                nc.tensor.matmul(
                                    cp1[:, :qw], V_tm[:nrow, kt, h0 + 1, :],
                                    ets_pend[kt][:nrow, 1, :qw],
                                    start=(kt == 0), stop=(kt == LB - 1))
                            normalize(h0, cp0, qw, ctx_sb)
                            normalize(h0 + 1, cp1, qw, ctx_sb)
                            ets_pend = ets_new

                        carry = emit_outproj(qo, qw, ctx_sb)

                    # drain the final chunk's output projection
                    for _ in carry:
                        pass

                    # ---- LN1 (in place on xmid) ----
                    layernorm(psW, "psW", rows, ap3, "sq", xmid, prms,
                              LN1G, LN1B, xmid)

                    # =================== PHASE B: FFN ===================
                    with tc.tile_pool(name=f"hB{l}", bufs=1) as fhp:
                        for (qo, qw) in QC:
                            h_sb = fhp.tile([P, FB, CW], bf16, tag="h")
                            for mf in range(FB):
                                hp_ = psA.tile([P, 2, 512], f32, tag="psA")
                                for kt in range(DB):
                                    nc.tensor.matmul(
                                        hp_[:, 0, :qw],
                                        w1_sb[:, kt, mf * P:(mf + 1) * P],
                                        xmid[:, kt, qo:qo + qw],
                                        start=(kt == 0), stop=(kt == DB - 1))
                                nc.scalar.activation(
                                    h_sb[:, mf, :qw], hp_[:, 0, :qw], AF.Gelu,
                                    bias=prms[:, B1S + mf:B1S + mf + 1])
                            for m in range(DB):
                                fp = psW.tile([P, 512], f32, tag="psW")
                                for kt in range(FB):
                                    nc.tensor.matmul(
                                        fp[:, :qw],
                                        w2_sb[:, kt, m * P:(m + 1) * P],
                                        h_sb[:, kt, :qw],
                                        start=(kt == 0), stop=(kt == FB - 1))
                                nc.vector.scalar_tensor_tensor(
                                    out=x[:, m, qo:qo + qw], in0=fp[:, :qw],
                                    scalar=prms[:, B2S + m:B2S + m + 1],
                                    in1=xmid[:, m, qo:qo + qw],
                                    op0=AO.add, op1=AO.add)

                        # ---- LN2 (in place on x) ----
                        layernorm(psW, "psW", rows, ap3, "sq", x, prms,
                                  LN2G, LN2B, x)

            # =================== transpose x -> out ===================
            # 4 feature-block transposes land in one PSUM tile, then one
            # [128, 512] cast-copy and one contiguous row-block DMA.
            with tc.tile_pool(name="psT", bufs=3, space="PSUM") as psT, \
                 tc.tile_pool(name="sbT", bufs=3) as sbT:
                for mt in range(LB):
                    nrow = P if mt < LB - 1 else L - (LB - 1) * P
                    tp = psT.tile([P, DB, P], bf16, tag="psT")
                    for m in range(DB):
                        nc.tensor.transpose(
                            tp[:nrow, m, :],
                            x[:, m, mt * P:mt * P + nrow],
                            ident[:])
                    ts = sbT.tile([P, DB * P], f32, tag="sbT")
                    nc.vector.tensor_copy(
                        ts[:nrow, :],
                        tp[:nrow, :, :].rearrange("p a b -> p (a b)"))
                    nc.sync.dma_start(
                        out_d[mt * P:mt * P + nrow, :], ts[:nrow, :])

    nc.compile()
    return nc


# ---------------------------------------------------------------- host side
def _prep_inputs(inputs):
    import ml_dtypes
    f = np.float32
    bf = ml_dtypes.bfloat16
    Wq, Wk, Wv, Wo = inputs["Wq"], inputs["Wk"], inputs["Wv"], inputs["Wo"]
    W1, W2 = inputs["W1"], inputs["W2"]

    def tb(a, perm):
        return np.ascontiguousarray(
            np.transpose(np.asarray(a, f), perm)).astype(bf)

    wqT = tb(Wq, (0, 2, 1))
    wkT = tb(Wk, (0, 2, 1))
    wvT = tb(Wv, (0, 2, 1))
    w1T = tb(W1, (0, 2, 1))
    w2T = tb(W2, (0, 2, 1))
    woTh = np.ascontiguousarray(np.transpose(
        np.asarray(Wo, f).reshape(NLAYER, D, H, DH), (0, 3, 2, 1))).astype(bf)

    def col(v):  # [NLAYER, D] -> [NLAYER, P, DB]
        return np.transpose(np.asarray(v, f).reshape(NLAYER, DB, P), (0, 2, 1))

    params = np.zeros((NLAYER, P, NPARAM), f)
    params[:, :, BQ:BQ + DB] = col(inputs["bq"])
    params[:, :, BK:BK + DB] = col(inputs["bk"])
    params[:, :, BV:BV + DB] = col(inputs["bv"])
    params[:, :, BO:BO + DB] = col(inputs["bo"])
    params[:, :, B2S:B2S + DB] = col(inputs["b2"])
    params[:, :, LN1G:LN1G + DB] = col(inputs["ln1_g"])
    params[:, :, LN1B:LN1B + DB] = col(inputs["ln1_b"])
    params[:, :, LN2G:LN2G + DB] = col(inputs["ln2_g"])
    params[:, :, LN2B:LN2B + DB] = col(inputs["ln2_b"])
    params[:, :, B1S:B1S + FB] = np.transpose(
        np.asarray(inputs["b1"], f).reshape(NLAYER, FB, P), (0, 2, 1))

    shared = {
        "cls": np.asarray(inputs["cls_token"], f).reshape(D, 1).astype(bf),
        "wqT": wqT, "wkT": wkT, "wvT": wvT, "woTh": woTh,
        "w1T": w1T, "w2T": w2T, "params": params,
        "bvrow": np.ascontiguousarray(np.asarray(inputs["bv"], f)),
    }
    e1 = np.asarray(inputs["embed1"], f).astype(bf)
    e2 = np.asarray(inputs["embed2"], f).astype(bf)
    in_maps = []
    for b in range(N_CORES):
        m = dict(shared)
        m["e1"] = np.ascontiguousarray(e1[b])
        m["e2"] = np.ascontiguousarray(e2[b])
        in_maps.append(m)
    return in_maps


def _run(inputs, trace=False, **kw):
    from concourse.bass_utils import run_bass_kernel_spmd

    if "nc" not in _CACHE:
        _CACHE["nc"] = _build_nc()
    nc = _CACHE["nc"]
    in_maps = _prep_inputs(inputs)
    res = run_bass_kernel_spmd(nc, in_maps, list(range(N_CORES)), trace=trace,
                               **kw)
    out = np.stack([res.results[b]["out"] for b in range(N_CORES)], axis=0)
    return out.astype(np.float32), res


def kernel(**inputs):
    out, _ = _run(inputs, trace=False)
    return out
